# revision 24
# baseline (speedup 1.0000x reference)
"""Trainium2 Bass kernel for single-query (decode-style) MultiHeadAttention.

Problem: x [32768, 1024] fp32; q is taken from x[:1] only, so the module is
    q = x[:1] @ Wq.T + bq                 (tiny)
    k = x @ Wk.T + bk ; v = x @ Wv.T + bv (huge: 2 x 68.7 GFLOP)
    out = out_proj(softmax(q k^T / sqrt(D)) v)

Key algebraic collapse (exact, decode-only): with a single query, per head h
    scores_h[j] = (q_h @ Wk_h) @ x_j / sqrt(D) + const_h
    attn_out_h  = Wv_h @ (sum_j p_hj x_j) / (sum_j p_hj) + bv_h
so the device never materializes K or V. Per-core device work:
    s   = R @ x_shard^T          (R = q@Wk/sqrt(D), [8, 1024], host-precomputed)
    p   = exp(s - shift)         (shift = per-head host constant; softmax-exact)
    U^T = p @ [x_shard | 1]      (gives both sum_j p x_j and l = sum_j p)
Host combines U/l across the 8 sequence shards (exact, shared shift) and
applies the tiny Wv / out_proj epilogue.

Sharding: K/V sequence dim split 8 ways (flash-decoding style per the hint);
q / projections replicated (folded into tiny host-side R and epilogue).
"""

import os
import sys

for _p in ("/opt/trn_rl_repo", "/root/.axon_site/_ro/trn_rl_repo"):
    if os.path.isdir(_p):
        sys.path.insert(0, _p)
        break

import numpy as np
import ml_dtypes

import concourse.bass as bass  # noqa: F401  (registers engine namespaces)
from concourse import bacc, mybir, tile
from concourse.bass_utils import run_bass_kernel_spmd
from concourse.masks import make_identity

L, E, H, D = 32768, 1024, 8, 128
NCORES = 8
LS = L // NCORES      # 4096 rows (keys) per core
NB = LS // 128        # 32 row blocks of 128
NK = E // 128         # 8 contraction chunks over E
NS = LS // 512        # 8 score chunks of 512 rows
BF16 = mybir.dt.bfloat16
F32 = mybir.dt.float32

_CACHE: dict = {}


def _build(loop_reps=None):
    nc = bacc.Bacc(
        "TRN2", target_bir_lowering=False, debug=False, num_devices=NCORES
    )
    # xn/xt are host-pre-linearized into the exact SBUF tile walk so every
    # big DMA is one fully-contiguous 8KB-per-partition run (measured ~12%
    # faster than 1-2KB strided runs on HW):
    #   xn[p, 4096*n + 1024*bl + e] = x[512*n + 128*bl + p, e]
    #   xt[p, 4096*n + 512*k + rl]  = x[512*n + rl, 128*k + p]
    xn = nc.dram_tensor("xn", [128, LS * E // 128], BF16, kind="ExternalInput").ap()
    xt = nc.dram_tensor("xt", [128, LS * E // 128], BF16, kind="ExternalInput").ap()
    # rt carries R^T split into a bf16 hi/lo pair (cols 0:8 hi, 8:16 lo);
    # both halves accumulate into the same PSUM bank, recovering ~fp32
    # precision for the score weights at negligible cost
    rt = nc.dram_tensor("rt", [E, 2 * H], BF16, kind="ExternalInput").ap()
    nshift = nc.dram_tensor("nshift", [H, 1], F32, kind="ExternalInput").ap()
    out = nc.dram_tensor("out", [H, E + 2], F32, kind="ExternalOutput").ap()

    rt_v = rt.rearrange("(k p) h -> p k h", p=128)   # [128, NK, 2H]

    with tile.TileContext(nc) as tc:
        with (
            tc.tile_pool(name="const", bufs=1) as cpool,
            tc.tile_pool(name="xn_res", bufs=1) as xn_pool,
            tc.tile_pool(name="xt_stream", bufs=5) as xt_pool,
            tc.tile_pool(name="pchunk", bufs=4) as p_pool,
            tc.tile_pool(name="ps_s", bufs=2, space="PSUM") as ps_s,
            tc.tile_pool(name="ps_t", bufs=2, space="PSUM") as ps_t,
            tc.tile_pool(name="ps_u", bufs=1, space="PSUM") as ps_u,
        ):

          def emit_body():
            # resident natural-layout x (bf16); a separate shared ones tile
            # turns the U matmul into [U | l]
            xn_all = xn_pool.tile([128, NB, E], BF16)
            ones2 = cpool.tile([128, 2], BF16)
            # DMA issue order = DMA execution order: tiny weights first, then
            # the first score-path tile so the s-pipeline starts immediately.
            rt_sb = cpool.tile([128, NK, 2 * H], BF16)
            nc.sync.dma_start(out=rt_sb[:], in_=rt_v)
            nb_sb = cpool.tile([H, 1], F32)
            nc.sync.dma_start(out=nb_sb[:], in_=nshift)

            # score-path (xt) transfers lead the natural-layout (xn) stream by
            # three chunks, so every chunk's s->exp->transpose chain completes
            # before its xn lands and the tail is just the last u-matmuls
            xt_tiles = {}
            for c in range(min(3, NS)):
                xt_tiles[c] = xt_pool.tile([128, NK, 512], BF16, name="xt_t")
                nc.sync.dma_start(
                    out=xt_tiles[c][:], in_=xt[:, 4096 * c : 4096 * (c + 1)]
                )

            ident = cpool.tile([128, 128], BF16)
            make_identity(nc, ident[:])
            nc.vector.memset(ones2[:], 1.0)

            pT_all = cpool.tile([128, NB, H], BF16)

            u0 = ps_u.tile([H, 512], F32)
            u1 = ps_u.tile([H, 512], F32)
            u2 = ps_u.tile([H, 2], F32)

            for n in range(NS):
                xt_t = xt_tiles.pop(n)
                # prefetch xt two chunks ahead, then this chunk's xn blocks;
                # the very last xn transfer is split so the closing u-matmuls
                # start as soon as possible
                if n + 3 < NS:
                    xt_tiles[n + 3] = xt_pool.tile(
                        [128, NK, 512], BF16, name="xt_t"
                    )
                    nc.sync.dma_start(
                        out=xt_tiles[n + 3][:],
                        in_=xt[:, 4096 * (n + 3) : 4096 * (n + 4)],
                    )
                if n < NS - 1:
                    nc.sync.dma_start(
                        out=xn_all[:, 4 * n : 4 * n + 4, :],
                        in_=xn[:, 4096 * n : 4096 * (n + 1)],
                    )
                else:
                    nc.sync.dma_start(
                        out=xn_all[:, 4 * n : 4 * n + 2, :],
                        in_=xn[:, 4096 * n : 4096 * n + 2048],
                    )
                    nc.sync.dma_start(
                        out=xn_all[:, 4 * n + 2 : 4 * n + 4, :],
                        in_=xn[:, 4096 * n + 2048 : 4096 * (n + 1)],
                    )
                # both R halves accumulate into the same PSUM region (exact
                # hi+lo sum comes free from PSUM accumulation)
                s_ps = ps_s.tile([H, 512], F32)
                for half in range(2):
                    for k in range(NK):
                        nc.tensor.matmul(
                            s_ps[:],
                            rt_sb[:, k, H * half : H * (half + 1)],
                            xt_t[:, k, :],
                            start=(half == 0 and k == 0),
                            stop=(half == 1 and k == NK - 1),
                        )
                p_chunk = p_pool.tile([H, 512], BF16)
                nc.scalar.activation(
                    p_chunk[:],
                    s_ps[:],
                    mybir.ActivationFunctionType.Exp,
                    bias=nb_sb[:],
                )
                for t in range(4):
                    jb = 4 * n + t
                    tr_ps = ps_t.tile([128, H], BF16)
                    nc.tensor.transpose(
                        tr_ps[:], p_chunk[:, 128 * t : 128 * (t + 1)], ident[:H, :H]
                    )
                    nc.vector.tensor_copy(pT_all[:, jb, :], tr_ps[:])
                for t in range(4):
                    jb = 4 * n + t
                    st, sp = (jb == 0), (jb == NB - 1)
                    nc.tensor.matmul(
                        u0[:], pT_all[:, jb, :], xn_all[:, jb, 0:512],
                        start=st, stop=sp,
                    )
                    nc.tensor.matmul(
                        u1[:], pT_all[:, jb, :], xn_all[:, jb, 512:1024],
                        start=st, stop=sp,
                    )
                    nc.tensor.matmul(
                        u2[:], pT_all[:, jb, :], ones2[:],
                        start=st, stop=sp,
                    )

            out_sb = cpool.tile([H, E + 2], F32)
            nc.vector.tensor_copy(out_sb[:, 0:512], u0[:])
            nc.scalar.copy(out_sb[:, 512:1024], u1[:])
            nc.vector.tensor_copy(out_sb[:, 1024:1026], u2[:])
            nc.sync.dma_start(out=out, in_=out_sb[:])

          if loop_reps:
              with tc.For_i(0, loop_reps, 1):
                  emit_body()
          else:
              emit_body()

    nc.compile()
    return nc


def _get_compiled():
    if "nc" not in _CACHE:
        _CACHE["nc"] = _build()
    return _CACHE["nc"]


def _prepare(inputs):
    x = np.asarray(inputs["x"], dtype=np.float32)
    w_in = np.asarray(inputs["in_proj_weight"], dtype=np.float32)
    b_in = np.asarray(inputs["in_proj_bias"], dtype=np.float32)

    Wq, Wk = w_in[:E], w_in[E : 2 * E]
    bq = b_in[:E]

    q = x[0].astype(np.float64) @ Wq.T.astype(np.float64) + bq  # [E]
    qh = q.reshape(H, D)
    # R[h] = q_h @ Wk_h / sqrt(D); scores_h[j] = R[h] . x_j (+ per-head const)
    Wk64 = Wk.astype(np.float64).reshape(H, D, E)
    R = np.einsum("hd,hde->he", qh, Wk64) / np.sqrt(D)  # [H, E]
    # Per-head shift ~= mean_j scores_h[j]; exact softmax invariance, keeps
    # exp() in a safe fp32 range whatever the input scale.
    shift = R @ x.astype(np.float64).mean(axis=0)  # [H]
    nshift = (-shift).astype(np.float32).reshape(H, 1)
    R_hi = R.astype(ml_dtypes.bfloat16)
    R_lo = (R - R_hi.astype(np.float64)).astype(ml_dtypes.bfloat16)
    rt_bf = np.ascontiguousarray(
        np.concatenate([R_hi, R_lo], axis=0).T
    )  # [E, 2H] bf16

    in_maps = []
    for c in range(NCORES):
        xs = x[c * LS : (c + 1) * LS].astype(ml_dtypes.bfloat16)
        # linearized tile-walk layouts (see _build comments)
        xn_lin = np.ascontiguousarray(
            xs.reshape(NS, 4, 128, E).transpose(2, 0, 1, 3)
        ).reshape(128, -1)
        xt_lin = np.ascontiguousarray(
            xs.reshape(NS, 512, NK, 128).transpose(3, 0, 2, 1)
        ).reshape(128, -1)
        in_maps.append(
            {"xn": xn_lin, "xt": xt_lin, "rt": rt_bf, "nshift": nshift}
        )
    return in_maps


def _epilogue(inputs, results):
    w_in = np.asarray(inputs["in_proj_weight"], dtype=np.float32)
    b_in = np.asarray(inputs["in_proj_bias"], dtype=np.float32)
    w_out = np.asarray(inputs["out_proj_weight"], dtype=np.float32)
    b_out = np.asarray(inputs["out_proj_bias"], dtype=np.float32)
    Wv = w_in[2 * E :]
    bv = b_in[2 * E :]

    Ug = np.zeros((H, E), np.float64)
    lg = np.zeros(H, np.float64)
    for c in range(NCORES):
        o = results[c]["out"].astype(np.float64)
        Ug += o[:, :E]
        lg += o[:, E]
    a = Ug / lg[:, None]  # [H, E] attention-weighted mean of x rows
    Wv64 = Wv.astype(np.float64).reshape(H, D, E)
    heads = np.einsum("hde,he->hd", Wv64, a) + bv.astype(np.float64).reshape(H, D)
    attn = heads.reshape(1, E)
    final = attn @ w_out.T.astype(np.float64) + b_out.astype(np.float64)
    return final.astype(np.float32)


def kernel(**inputs) -> np.ndarray:
    nc = _get_compiled()
    in_maps = _prepare(inputs)
    res = run_bass_kernel_spmd(nc, in_maps, list(range(NCORES))).results
    return _epilogue(inputs, res)


# revision 28
# speedup vs baseline: 1.1846x; 1.1846x over previous
"""Trainium2 Bass kernel for single-query (decode-style) MultiHeadAttention.

Problem: x [32768, 1024] fp32; q is taken from x[:1] only, so the module is
    q = x[:1] @ Wq.T + bq                 (tiny)
    k = x @ Wk.T + bk ; v = x @ Wv.T + bv (huge: 2 x 68.7 GFLOP)
    out = out_proj(softmax(q k^T / sqrt(D)) v)

Key algebraic collapse (exact, decode-only): with a single query, per head h
    scores_h[j] = (q_h @ Wk_h) @ x_j / sqrt(D) + const_h
    attn_out_h  = Wv_h @ (sum_j p_hj x_j) / (sum_j p_hj) + bv_h
so the device never materializes K or V. Per-core device work:
    s   = R @ x_shard^T          (R = q@Wk/sqrt(D), [8, 1024], host-precomputed)
    p   = exp(s - shift)         (shift = per-head host constant; softmax-exact)
    U^T = p @ [x_shard | 1]      (gives both sum_j p x_j and l = sum_j p)
Host combines U/l across the 8 sequence shards (exact, shared shift) and
applies the tiny Wv / out_proj epilogue.

Sharding: K/V sequence dim split 8 ways (flash-decoding style per the hint);
q / projections replicated (folded into tiny host-side R and epilogue).
"""

import os
import sys

for _p in ("/opt/trn_rl_repo", "/root/.axon_site/_ro/trn_rl_repo"):
    if os.path.isdir(_p):
        sys.path.insert(0, _p)
        break

import numpy as np
import ml_dtypes

import concourse.bass as bass  # noqa: F401  (registers engine namespaces)
from concourse import bacc, mybir, tile
from concourse.bass_utils import run_bass_kernel_spmd
from concourse.masks import make_identity

L, E, H, D = 32768, 1024, 8, 128
NCORES = 8
LS = L // NCORES      # 4096 rows (keys) per core
NB = LS // 128        # 32 row blocks of 128
NK = E // 128         # 8 contraction chunks over E
NS = LS // 512        # 8 score chunks of 512 rows
BF16 = mybir.dt.bfloat16
F32 = mybir.dt.float32

_CACHE: dict = {}


def _build(loop_reps=None):
    nc = bacc.Bacc(
        "TRN2", target_bir_lowering=False, debug=False, num_devices=NCORES
    )
    # xn/xt are host-pre-linearized into the exact SBUF tile walk so every
    # big DMA is one fully-contiguous 8KB-per-partition run (measured ~12%
    # faster than 1-2KB strided runs on HW):
    #   xn[p, 4096*n + 1024*bl + e] = x[512*n + 128*bl + p, e]
    #   xt[p, 4096*n + 512*k + rl]  = x[512*n + rl, 128*k + p]
    xn = nc.dram_tensor("xn", [128, LS * E // 128], BF16, kind="ExternalInput").ap()
    xt = nc.dram_tensor("xt", [128, LS * E // 128], BF16, kind="ExternalInput").ap()
    # rt carries R^T split into a bf16 hi/lo pair (cols 0:8 hi, 8:16 lo);
    # both halves accumulate into the same PSUM bank, recovering ~fp32
    # precision for the score weights at negligible cost
    # rt linearized likewise: rt[p, 16*k + c] = R_pair^T[128*k + p, c]
    rt = nc.dram_tensor("rt", [128, NK * 2 * H], BF16, kind="ExternalInput").ap()
    nshift = nc.dram_tensor("nshift", [H, 1], F32, kind="ExternalInput").ap()
    out = nc.dram_tensor("out", [H, E + 2], F32, kind="ExternalOutput").ap()

    with tile.TileContext(nc) as tc:
        with (
            tc.tile_pool(name="const", bufs=1) as cpool,
            tc.tile_pool(name="xn_res", bufs=1) as xn_pool,
            tc.tile_pool(name="xt_stream", bufs=5) as xt_pool,
            tc.tile_pool(name="pchunk", bufs=4) as p_pool,
            tc.tile_pool(name="ps_s", bufs=2, space="PSUM") as ps_s,
            tc.tile_pool(name="ps_t", bufs=2, space="PSUM") as ps_t,
            tc.tile_pool(name="ps_u", bufs=1, space="PSUM") as ps_u,
        ):

          def emit_body():
            # resident natural-layout x (bf16); a separate shared ones tile
            # turns the U matmul into [U | l]
            xn_all = xn_pool.tile([128, NB, E], BF16)
            ones2 = cpool.tile([128, 2], BF16)
            # DMA issue order = DMA execution order. The first score-path tile
            # goes absolutely first: descriptor generations serialize on the
            # shared HWDGE resource, and the first s-matmul is gated by xt0's
            # arrival anyway — the tiny rt/nshift transfers slot in behind it.
            # xt transfers lead the natural-layout (xn) stream by three chunks,
            # so every chunk's s->exp->transpose chain completes before its xn
            # lands and the tail is just the last u-matmuls.
            xt_tiles = {}
            xt_tiles[0] = xt_pool.tile([128, NK, 512], BF16, name="xt_t")
            nc.sync.dma_start(out=xt_tiles[0][:], in_=xt[:, 0:4096])

            rt_sb = cpool.tile([128, NK, 2 * H], BF16)
            nc.sync.dma_start(out=rt_sb[:], in_=rt)
            nb_sb = cpool.tile([H, 1], F32)
            nc.sync.dma_start(out=nb_sb[:], in_=nshift)

            for c in range(1, min(3, NS)):
                xt_tiles[c] = xt_pool.tile([128, NK, 512], BF16, name="xt_t")
                nc.sync.dma_start(
                    out=xt_tiles[c][:], in_=xt[:, 4096 * c : 4096 * (c + 1)]
                )

            ident = cpool.tile([128, 128], BF16)
            make_identity(nc, ident[:])
            nc.vector.memset(ones2[:], 1.0)

            pT_all = cpool.tile([128, NB, H], BF16)

            u0 = ps_u.tile([H, 512], F32)
            u1 = ps_u.tile([H, 512], F32)
            u2 = ps_u.tile([H, 2], F32)

            for n in range(NS):
                xt_t = xt_tiles.pop(n)
                # prefetch xt two chunks ahead, then this chunk's xn blocks;
                # the very last xn transfer is split so the closing u-matmuls
                # start as soon as possible
                if n + 3 < NS:
                    xt_tiles[n + 3] = xt_pool.tile(
                        [128, NK, 512], BF16, name="xt_t"
                    )
                    nc.sync.dma_start(
                        out=xt_tiles[n + 3][:],
                        in_=xt[:, 4096 * (n + 3) : 4096 * (n + 4)],
                    )
                if n < NS - 1:
                    nc.sync.dma_start(
                        out=xn_all[:, 4 * n : 4 * n + 4, :],
                        in_=xn[:, 4096 * n : 4096 * (n + 1)],
                    )
                else:
                    nc.sync.dma_start(
                        out=xn_all[:, 4 * n : 4 * n + 2, :],
                        in_=xn[:, 4096 * n : 4096 * n + 2048],
                    )
                    nc.sync.dma_start(
                        out=xn_all[:, 4 * n + 2 : 4 * n + 4, :],
                        in_=xn[:, 4096 * n + 2048 : 4096 * (n + 1)],
                    )
                # both R halves accumulate into the same PSUM region (exact
                # hi+lo sum comes free from PSUM accumulation)
                s_ps = ps_s.tile([H, 512], F32)
                for half in range(2):
                    for k in range(NK):
                        nc.tensor.matmul(
                            s_ps[:],
                            rt_sb[:, k, H * half : H * (half + 1)],
                            xt_t[:, k, :],
                            start=(half == 0 and k == 0),
                            stop=(half == 1 and k == NK - 1),
                        )
                p_chunk = p_pool.tile([H, 512], BF16)
                nc.scalar.activation(
                    p_chunk[:],
                    s_ps[:],
                    mybir.ActivationFunctionType.Exp,
                    bias=nb_sb[:],
                )
                for t in range(4):
                    jb = 4 * n + t
                    tr_ps = ps_t.tile([128, H], BF16)
                    nc.tensor.transpose(
                        tr_ps[:], p_chunk[:, 128 * t : 128 * (t + 1)], ident[:H, :H]
                    )
                    nc.vector.tensor_copy(pT_all[:, jb, :], tr_ps[:])
                for t in range(4):
                    jb = 4 * n + t
                    st, sp = (jb == 0), (jb == NB - 1)
                    nc.tensor.matmul(
                        u0[:], pT_all[:, jb, :], xn_all[:, jb, 0:512],
                        start=st, stop=sp,
                    )
                    nc.tensor.matmul(
                        u1[:], pT_all[:, jb, :], xn_all[:, jb, 512:1024],
                        start=st, stop=sp,
                    )
                    nc.tensor.matmul(
                        u2[:], pT_all[:, jb, :], ones2[:],
                        start=st, stop=sp,
                    )

            out_sb = cpool.tile([H, E + 2], F32)
            nc.vector.tensor_copy(out_sb[:, 0:512], u0[:])
            nc.scalar.copy(out_sb[:, 512:1024], u1[:])
            nc.vector.tensor_copy(out_sb[:, 1024:1026], u2[:])
            nc.sync.dma_start(out=out, in_=out_sb[:])

          if loop_reps:
              with tc.For_i(0, loop_reps, 1):
                  emit_body()
          else:
              emit_body()

    nc.compile()
    return nc


def _get_compiled():
    if "nc" not in _CACHE:
        _CACHE["nc"] = _build()
    return _CACHE["nc"]


def _prepare(inputs):
    x = np.asarray(inputs["x"], dtype=np.float32)
    w_in = np.asarray(inputs["in_proj_weight"], dtype=np.float32)
    b_in = np.asarray(inputs["in_proj_bias"], dtype=np.float32)

    Wq, Wk = w_in[:E], w_in[E : 2 * E]
    bq = b_in[:E]

    q = x[0].astype(np.float64) @ Wq.T.astype(np.float64) + bq  # [E]
    qh = q.reshape(H, D)
    # R[h] = q_h @ Wk_h / sqrt(D); scores_h[j] = R[h] . x_j (+ per-head const)
    Wk64 = Wk.astype(np.float64).reshape(H, D, E)
    R = np.einsum("hd,hde->he", qh, Wk64) / np.sqrt(D)  # [H, E]
    # Per-head shift ~= mean_j scores_h[j]; exact softmax invariance, keeps
    # exp() in a safe fp32 range whatever the input scale.
    shift = R @ x.astype(np.float64).mean(axis=0)  # [H]
    nshift = (-shift).astype(np.float32).reshape(H, 1)
    R_hi = R.astype(ml_dtypes.bfloat16)
    R_lo = (R - R_hi.astype(np.float64)).astype(ml_dtypes.bfloat16)
    RT = np.concatenate([R_hi, R_lo], axis=0).T  # [E, 2H] bf16
    rt_bf = np.ascontiguousarray(
        RT.reshape(NK, 128, 2 * H).transpose(1, 0, 2)
    ).reshape(128, -1)  # [128, NK*2H]

    in_maps = []
    for c in range(NCORES):
        xs = x[c * LS : (c + 1) * LS].astype(ml_dtypes.bfloat16)
        # linearized tile-walk layouts (see _build comments)
        xn_lin = np.ascontiguousarray(
            xs.reshape(NS, 4, 128, E).transpose(2, 0, 1, 3)
        ).reshape(128, -1)
        xt_lin = np.ascontiguousarray(
            xs.reshape(NS, 512, NK, 128).transpose(3, 0, 2, 1)
        ).reshape(128, -1)
        in_maps.append(
            {"xn": xn_lin, "xt": xt_lin, "rt": rt_bf, "nshift": nshift}
        )
    return in_maps


def _epilogue(inputs, results):
    w_in = np.asarray(inputs["in_proj_weight"], dtype=np.float32)
    b_in = np.asarray(inputs["in_proj_bias"], dtype=np.float32)
    w_out = np.asarray(inputs["out_proj_weight"], dtype=np.float32)
    b_out = np.asarray(inputs["out_proj_bias"], dtype=np.float32)
    Wv = w_in[2 * E :]
    bv = b_in[2 * E :]

    Ug = np.zeros((H, E), np.float64)
    lg = np.zeros(H, np.float64)
    for c in range(NCORES):
        o = results[c]["out"].astype(np.float64)
        Ug += o[:, :E]
        lg += o[:, E]
    a = Ug / lg[:, None]  # [H, E] attention-weighted mean of x rows
    Wv64 = Wv.astype(np.float64).reshape(H, D, E)
    heads = np.einsum("hde,he->hd", Wv64, a) + bv.astype(np.float64).reshape(H, D)
    attn = heads.reshape(1, E)
    final = attn @ w_out.T.astype(np.float64) + b_out.astype(np.float64)
    return final.astype(np.float32)


def kernel(**inputs) -> np.ndarray:
    nc = _get_compiled()
    in_maps = _prepare(inputs)
    res = run_bass_kernel_spmd(nc, in_maps, list(range(NCORES))).results
    return _epilogue(inputs, res)
